# revision 13
# baseline (speedup 1.0000x reference)
"""Trainium2 Bass kernel for the ConvolutionalOverlap problem.

Reference computation (x: [2, 1, 256, 256] f32, w1/w2 scalar):
    out[b, i, h, w] = w1 * x[b, 0, h, w - (i+1)//2] + w2 * x[b, 0, h, w + (i+2)//2]
    (terms outside [0, W) are zero), out shape [2, 256, 256, 256].

Strategy (pure SPMD across 8 cores, identical program, different data):
  - Flatten (b, h) into 512 rows; shard 64 rows per core.
  - On each core, duplicate the 64 rows onto both SBUF partition halves:
    partitions 0..63 compute output columns w in [0, 128) and hold
    x zero-padded by 128 on the left; partitions 64..127 compute
    w in [128, 256) and hold x unshifted (zero-padded on the right).
    With that per-half staging, one free-dim access pattern serves all
    128 partitions, and the zero padding implements the boundary masks.
  - out[ch] = shift(w1*x, s1(ch)) + shift(w2*x, -s2(ch)).  s1/s2 are
    affine in the channel pair index, so one DVE instruction per
    (group, parity) computes a whole channel group.

Schedule (HW-measured on this pod, per core: ~0.2 MB in, 16 MB out,
HBM-write roofline ~46 us per 16 MB with 7 alternating out-DMAs):
  - critical path = in-DMA chain (~2.9 us) -> first-group compute ->
    first out-DMA -> 46 us write stream -> completion receipt.
  - group 0 (8 ch) is computed WITHOUT the ACT staging hop: DVE itself
    stages X2d = w2*x over the 131 columns group 0 needs
    (tensor_scalar), then 2 scalar_tensor_tensor ops -> first out-DMA
    launches as early as possible.
  - ACT (idle otherwise) concurrently stages X1 = w1*x and X2 = w2*x;
    groups 1..6 are plain DVE tensor_tensor adds (1 elem/cycle/lane;
    ~34 us total DVE, comfortably ahead of the 46 us stream).
  - 7 graduated out-DMA groups [12,16,24,32,48,64,60] alternate between
    the two HWDGE rings (SP/ACT).  Sizes chosen so each group's DVE
    completion (0.131 us/ch) stays ahead of the write stream's arrival
    at that group (0.18 us/ch) -- no ring-refill bubbles -- while
    keeping the first out-DMA launch early.  1 in-DMA + 7 out-DMAs ==
    the max 8 DMAHW sem lanes this walrus codegen path supports.
    (Alternatives measured on HW, barrier-loop per-iteration: baseline
    6-group stt kernel 59.0 us; [8,16,32,48,56,48,48] 57.6 us; SWDGE
    input + 8 groups 56.2 us; this config 55.8 us.)

build_nc(loop_k=K) additionally wraps the identical body in a per-engine
hardware Fori loop with a full inter-iteration barrier (iteration i+1's
input DMA waits for every iteration-i output DMA), so per-iteration time
on HW equals the single-shot critical path -- used by bench_loop.py.
The default (loop_k=None) straight-line program is what kernel() runs.
"""

import sys

import numpy as np

if "/opt/trn_rl_repo" not in sys.path:
    sys.path.insert(0, "/opt/trn_rl_repo")

import concourse.bass as bass
import concourse.mybir as mybir
from concourse.ap import AP

F32 = mybir.dt.float32
P = 128          # SBUF partitions
W = 256          # spatial width == number of output channels
WH = W // 2      # output columns per partition half
XW = 388         # padded x width: j in [0, 384); cols 384/385 hold w1/w2
ROWS = 512       # B * H
NCORES = 8
RPC = ROWS // NCORES  # rows per core (64)
# Channel group sizes (sum 256).  Graduated: small first group -> first
# output DMA launches early; the rest sized so the DVE stays ahead of the
# write stream.  1 in-DMA + len(GROUPS) out-DMAs must stay <= 8 (DMAHW
# sem lanes; a 9th DMA wraps onto lane 0 adding a 2nd sync-wait, which
# this walrus codegen path rejects).
GROUPS = [12, 16, 24, 32, 48, 64, 60]
IN_SWDGE = False  # issue the input DMA from GPSIMD (SWDGE) instead of SP:
                  # frees an HWDGE DMAHW sem lane for an 8th output group

_nc_cache = None


def _sub(tile_ap, off, dims):
    """AP over `tile_ap`'s tensor: all 128 partitions, custom free dims."""
    if not isinstance(tile_ap, AP):
        tile_ap = tile_ap[:]
    part = list(tile_ap.ap)[0]
    return AP(
        tile_ap.tensor,
        tile_ap.offset + off,
        [list(part)] + [list(d) for d in dims],
    )


class _Cum:
    """Cumulative semaphore-wait target for one (engine, sem) pair.

    Straight-line mode tracks the target as a Python int (identical
    codegen to hand-written constants); loop mode keeps it in an engine
    register updated with reg_add so targets grow across iterations.
    """

    def __init__(self, eng, sem, looped, name):
        self.eng, self.sem, self.looped = eng, sem, looped
        self.v = 0
        if looped:
            self.r = eng.alloc_register(name)
            eng.reg_mov(self.r, 0)

    def bump(self, n):
        self.v += n
        if self.looped:
            self.eng.reg_add(self.r, self.r, n)

    def wait(self):
        self.eng.wait_ge(self.sem, self.r if self.looped else self.v)

    def wait_inc(self, n):
        self.bump(n)
        self.wait()


def build_nc(loop_k=None):
    """Raw Bass (no TileContext): explicit sems, <=1 sync-wait per
    instruction (this walrus codegen path rejects multi-wait instructions,
    including Tile's tail drain)."""
    nc = bass.Bass(trn_type="TRN2")
    xp = nc.dram_tensor("xp", [P, XW], F32, kind="ExternalInput")
    out = nc.dram_tensor("out", [P, W * WH], F32, kind="ExternalOutput")
    looped = loop_k is not None
    ng = len(GROUPS)

    from contextlib import ExitStack

    with ExitStack() as ctx:
        x2d_w = GROUPS[0] // 2 + 128  # g0's in1 window: cols [129, ...)
        Xp = ctx.enter_context(nc.sbuf_tensor("Xp", [P, XW], F32))
        X1 = ctx.enter_context(nc.sbuf_tensor("X1", [P, 256], F32))
        X2 = ctx.enter_context(nc.sbuf_tensor("X2", [P, 255], F32))
        X2d = ctx.enter_context(nc.sbuf_tensor("X2d", [P, x2d_w], F32))
        Os = [
            ctx.enter_context(nc.sbuf_tensor(f"O{g}", [P, n * WH], F32))
            for g, n in enumerate(GROUPS)
        ]
        sem_in = ctx.enter_context(nc.semaphore("sem_in"))
        sem_x2d = ctx.enter_context(nc.semaphore("sem_x2d"))
        sem_stage = ctx.enter_context(nc.semaphore("sem_stage"))
        sem_dve = ctx.enter_context(nc.semaphore("sem_dve"))
        sem_out = ctx.enter_context(nc.semaphore("sem_out"))

        sp, act, dve = nc.sync, nc.scalar, nc.vector
        in_eng = nc.gpsimd if IN_SWDGE else sp
        c_act_in = _Cum(act, sem_in, looped, "c_act_in")
        c_bar_out = _Cum(in_eng, sem_out, looped, "c_bar_out")
        c_dve_in = _Cum(dve, sem_in, looped, "c_dve_in")
        c_dve_x2d = _Cum(dve, sem_x2d, looped, "c_dve_x2d")
        c_dve_stage = _Cum(dve, sem_stage, looped, "c_dve_stage")
        c_sp_dve = _Cum(sp, sem_dve, looped, "c_sp_dve")
        c_act_dve = _Cum(act, sem_dve, looped, "c_act_dve")
        c_sp_out = _Cum(sp, sem_out, looped, "c_sp_out")
        c_act_out = _Cum(act, sem_out, looped, "c_act_out")

        W1 = Xp[:, 384:385]
        W2 = Xp[:, 385:386]
        Copy = mybir.ActivationFunctionType.Copy

        def body():
            if looped:
                # Inter-iteration barrier: the input DMA may not overwrite
                # Xp until every previous-iteration output DMA (which
                # transitively orders all compute) has completed.
                c_bar_out.wait()

            # Load the packed input (x rows, padded + duplicated, w1/w2).
            in_eng.dma_start(out=Xp[:], in_=xp[:]).then_inc(sem_in, 16)

            # DVE fast path for group 0 (no ACT hop): stage the 131-col
            # X2d = w2*x window, then 2 stt ops -> first out-DMA data.
            c_dve_in.wait_inc(16)
            dve.tensor_scalar(
                X2d[:], Xp[:, 129:129 + x2d_w], W2, None, mybir.AluOpType.mult
            ).then_inc(sem_x2d, 1)
            # same-engine RAW: DVE pipelines back-to-back instructions, so
            # the stt reads of X2d must wait for the staging write to land.
            c_dve_x2d.wait_inc(1)
            n0 = GROUPS[0]
            last = None
            for parity in range(2):
                in0 = _sub(Xp, 128 - parity, [(-1, n0 // 2), (1, WH)])
                in1 = _sub(X2d, 0, [(1, n0 // 2), (1, WH)])
                o = _sub(Os[0], parity * WH, [(2 * WH, n0 // 2), (1, WH)])
                last = dve.scalar_tensor_tensor(
                    o, in0, W1, in1,
                    mybir.AluOpType.mult, mybir.AluOpType.add,
                )
            last.then_inc(sem_dve, 1)

            # ACT (concurrently): stage X1 = w1*x (cols 0..256) and
            # X2 = w2*x (Xp cols 129..384; X2[j] == w2*Xp[j+129]).
            c_act_in.wait_inc(16)
            act.activation(X2[:], Xp[:, 129:384], Copy, 0.0, W2).then_inc(
                sem_stage, 1
            )
            act.activation(X1[:], Xp[:, 0:256], Copy, 0.0, W1).then_inc(
                sem_stage, 1
            )

            # Groups 1+: plain DVE tensor_tensor adds.
            c_dve_stage.wait_inc(2)
            c0 = GROUPS[0]
            for g, n in enumerate(GROUPS[1:], start=1):
                base, pairs = c0 // 2, n // 2
                for parity in range(2):
                    in0 = _sub(X1, 128 - base - parity, [(-1, pairs), (1, WH)])
                    in1 = _sub(X2, base, [(1, pairs), (1, WH)])
                    o = _sub(Os[g], parity * WH, [(2 * WH, pairs), (1, WH)])
                    last = dve.tensor_tensor(o, in0, in1, mybir.AluOpType.add)
                last.then_inc(sem_dve, 1)
                c0 += n

            # Out DMAs alternate between the two HWDGE rings (SP / ACT);
            # each waits on its producer's sem (1 wait per instruction).
            c0 = 0
            for g, n in enumerate(GROUPS):
                eng, cum = (sp, c_sp_dve) if g % 2 == 0 else (act, c_act_dve)
                # group g's DVE pair raises sem_dve to (iter*ng + g + 1)
                cum.wait_inc(1 if g == 0 else 2)
                eng.dma_start(
                    out=out[:, c0 * WH:(c0 + n) * WH], in_=Os[g][:]
                ).then_inc(sem_out, 16)
                c0 += n
            if ng % 2 == 0:
                c_sp_dve.bump(1)  # resync to ng per iteration
            else:
                c_act_dve.bump(1)
            c_sp_out.bump(16 * ng)
            c_act_out.bump(16 * ng)
            c_bar_out.bump(16 * ng)

        if looped:
            loop_engines = (
                mybir.EngineType.SP,
                mybir.EngineType.Activation,
                mybir.EngineType.DVE,
            )
            if IN_SWDGE:
                loop_engines += (mybir.EngineType.Pool,)
            with nc.Fori(0, loop_k, engines=loop_engines):
                body()
        else:
            body()

        # Each issuing engine waits for all out-DMA completions so the
        # NEFF doesn't finish with DMAs in flight.
        c_sp_out.wait()
        c_act_out.wait()
    return nc


def get_nc():
    global _nc_cache
    if _nc_cache is None:
        _nc_cache = build_nc()
    return _nc_cache


def prep_in_maps(x, w1, w2):
    """Shard + stage inputs for the 8 cores (host-side data movement only)."""
    x2 = np.ascontiguousarray(np.asarray(x, dtype=np.float32)[:, 0]).reshape(
        ROWS, W
    )
    w1v = np.float32(np.asarray(w1).reshape(-1)[0])
    w2v = np.float32(np.asarray(w2).reshape(-1)[0])
    in_maps = []
    for c in range(NCORES):
        rows = x2[c * RPC:(c + 1) * RPC]  # [64, 256]
        xp = np.zeros((P, XW), dtype=np.float32)
        xp[:RPC, 128:128 + W] = rows      # half 0: columns w in [0, 128)
        xp[RPC:, 0:W] = rows              # half 1: columns w in [128, 256)
        xp[:, 384] = w1v
        xp[:, 385] = w2v
        in_maps.append({"xp": xp})
    return in_maps


def gather(outs):
    """Reassemble per-core [128, 256*128] outputs into [2, 256, 256, 256]."""
    parts = []
    for oc in outs:
        oc = np.asarray(oc).reshape(2, RPC, W, WH)  # [whalf, row, ch, w']
        parts.append(oc.transpose(1, 2, 0, 3).reshape(RPC, W, W))
    out_rows = np.concatenate(parts, axis=0)        # [512 rows, ch, w]
    return np.ascontiguousarray(
        out_rows.reshape(2, 256, W, W).transpose(0, 2, 1, 3)
    )


def kernel(x, w1, w2, _run_kwargs=None):
    from concourse.bass_utils import run_bass_kernel_spmd

    nc = get_nc()
    in_maps = prep_in_maps(x, w1, w2)
    kwargs = _run_kwargs or {}
    res = run_bass_kernel_spmd(nc, in_maps, core_ids=list(range(NCORES)), **kwargs)
    out = gather([r["out"] for r in res.results])
    if kwargs:
        kernel.last_results = res
    return out
